# revision 10
# baseline (speedup 1.0000x reference)
"""Trainium2 Bass kernel for unnormalized multi-head attention over W axis.

Reference computation (per batch b, row h):
    Qp[w,o] = sum_c wq[o,c] q[b,c,h,w] + bq[o]      (1x1 conv projections)
    Kp, Vp analogous
    attn[w,x] = sum_o Qp[w,o] Kp[x,o]
    out[b,o,h,w] = sum_x attn[w,x] Vp[x,o] + q[b,o,h,w]

Sharding: data-parallel over batch B=32 across 8 cores (4 batches/core),
weights replicated.

Layout: tiles of 16 attention rows are loaded "stacked": SBUF partitions
0-63 hold channels of rows h0..h0+7 ("lo"), partitions 64-127 hold rows
h0+8..h0+15 ("hi").  Every matmul runs as a lo/hi pair packed into
disjoint halves of the 128x128 PE array, doubling TensorE throughput and
keeping every PSUM->SBUF copy a full-width 128-partition op.

Matmul operands are fp16 (PSUM accumulation stays fp32): 16-bit matmuls
get a separate LDWEIGHTS instruction that pipelines with MATMULs through
the PE reorder window, roughly halving TensorE time vs fp32 self-loading
matmuls; inputs are cast fp32->fp16 by the (gpsimd) DMA itself.

HW bug workaround: a stationary loaded to PE rows 64-127 with M=128
(tile_position (64,0), all PE columns) crashes the exec unit, for any
dtype.  All "hi" matmuls with M=128 are split into two M=64 matmuls.
"""

from contextlib import ExitStack

import numpy as np

import concourse.bass as bass
import concourse.tile as tile
from concourse import bacc, mybir
from concourse.bass_utils import run_bass_kernel_spmd
from concourse.masks import make_identity

FP32 = mybir.dt.float32
FP16 = mybir.dt.float16
AF = mybir.ActivationFunctionType

B, C, H, W = 32, 64, 128, 128
N_CORES = 8
BP = B // N_CORES      # batches per core
GL = 16                # attention rows per load tile
NJ = GL // 2           # rows per half within a load tile
G = 4                  # rows per compute sub-group (2 pairs)


def build_nc(bp: int = BP, h: int = H, reps: int = 1) -> bass.Bass:
    """reps>1 repeats the whole (idempotent) computation for timing:
    per-iteration time = (wall[reps=R] - wall[reps=1]) / (R - 1)."""
    nc = bacc.Bacc(None, name="mha_w")

    q_d = nc.dram_tensor("q", [bp, C, h, W], FP32, kind="ExternalInput")
    k_d = nc.dram_tensor("k", [bp, C, h, W], FP32, kind="ExternalInput")
    v_d = nc.dram_tensor("v", [bp, C, h, W], FP32, kind="ExternalInput")
    wq_d = nc.dram_tensor("wq", [C, C], FP32, kind="ExternalInput")
    bq_d = nc.dram_tensor("bq", [C], FP32, kind="ExternalInput")
    wk_d = nc.dram_tensor("wk", [C, C], FP32, kind="ExternalInput")
    bk_d = nc.dram_tensor("bk", [C], FP32, kind="ExternalInput")
    wv_d = nc.dram_tensor("wv", [C, C], FP32, kind="ExternalInput")
    bv_d = nc.dram_tensor("bv", [C], FP32, kind="ExternalInput")
    out_d = nc.dram_tensor("out", [bp, C, h, W], FP32, kind="ExternalOutput")

    n_gl = h // GL
    hj = NJ

    with tile.TileContext(nc) as tc, ExitStack() as ctx:
        consts = ctx.enter_context(tc.tile_pool(name="consts", bufs=1))
        xin = ctx.enter_context(tc.tile_pool(name="xin", bufs=3))
        mid = ctx.enter_context(tc.tile_pool(name="mid", bufs=2))
        outs = ctx.enter_context(tc.tile_pool(name="outs", bufs=2))
        psum = ctx.enter_context(tc.tile_pool(name="psum", bufs=2, space="PSUM"))

        # ---- constants -------------------------------------------------
        i64 = consts.tile([C, C], FP32, tag="i64")
        make_identity(nc, i64)

        # transposed fp16 conv weights, duplicated on both partition halves
        wT = {}
        for name, wd in (("wq", wq_d), ("wk", wk_d), ("wv", wv_d)):
            w_nat = consts.tile([C, C], FP32, tag=f"{name}_nat")
            nc.sync.dma_start(out=w_nat[:], in_=wd[:, :])
            w_ps = psum.tile([128, C], FP32, tag="PROJ")
            nc.tensor.matmul(w_ps[0:C, :], lhsT=w_nat[:], rhs=i64[:],
                             start=True, stop=True)
            nc.tensor.matmul(w_ps[C:128, :], lhsT=w_nat[:], rhs=i64[:],
                             start=True, stop=True)
            wt = consts.tile([128, C], FP16, tag=f"{name}T")
            nc.vector.tensor_copy(wt[:], w_ps[:])
            wT[name] = wt

        # per-partition biases for q/k projections, on both halves (fp32)
        bias2 = {}
        for name, bd in (("bq", bq_d), ("bk", bk_d)):
            bt = consts.tile([128, 1], FP32, tag=f"{name}2")
            nc.sync.dma_start(out=bt[0:C, :], in_=bd[:].unsqueeze(1))
            nc.sync.dma_start(out=bt[C:128, :], in_=bd[:].unsqueeze(1))
            bias2[name] = bt

        # bv broadcast over all 128 partitions: [128, C] fp32
        bv_bc = consts.tile([128, C], FP32, tag="bv_bc")
        bv_bcast_ap = bass.AP(tensor=bv_d[:].tensor, offset=0,
                              ap=[[0, 128], [1, C]])
        nc.sync.dma_start(out=bv_bc[:], in_=bv_bcast_ap)

        # ---- main loop -------------------------------------------------
        def stacked(dram, b, h0):
            # [C, GL, W] -> (half, c, j, w), contiguous (j w) runs of 4KB
            a = dram[b, :, h0:h0 + GL, :]
            return a.rearrange("c (p j) w -> c p j w", p=2).transpose([1, 0, 2, 3])

        for rep in range(reps):
          for b in range(bp):
            for gl in range(n_gl):
                h0 = gl * GL
                q_ld = xin.tile([128, hj, W], FP16, tag="q_ld")
                k_ld = xin.tile([128, hj, W], FP16, tag="k_ld")
                v_ld = xin.tile([128, hj, W], FP16, tag="v_ld")
                # gpsimd DMA casts fp32 (DRAM) -> fp16 (SBUF)
                nc.gpsimd.dma_start(out=q_ld[:], in_=stacked(q_d, b, h0))
                nc.gpsimd.dma_start(out=k_ld[:], in_=stacked(k_d, b, h0))
                nc.gpsimd.dma_start(out=v_ld[:], in_=stacked(v_d, b, h0))

                out_ld = outs.tile([128, hj, W], FP32, tag="out_ld")

                for gc in range(hj // 2):  # sub-groups of 2 pairs = 4 rows
                    j0 = 2 * gc
                    # --- projections -> PSUM ---
                    proj = psum.tile([128, 4 * W], FP32, tag="PROJ")
                    vp_ps = psum.tile([128, 4, C], FP32, tag="VP")
                    for lo in range(2):
                        pr = slice(C * lo, C * (lo + 1))
                        # q/k: both row-pairs in one N=256 matmul
                        nc.tensor.matmul(
                            proj[pr, 0:2 * W],
                            lhsT=wT["wq"][pr, :], rhs=q_ld[pr, j0:j0 + 2, :],
                            start=True, stop=True)
                        nc.tensor.matmul(
                            proj[pr, 2 * W:4 * W],
                            lhsT=wT["wk"][pr, :], rhs=k_ld[pr, j0:j0 + 2, :],
                            start=True, stop=True)
                    for jj in range(2):
                        j = j0 + jj
                        # v projection, transposed: [x, c] per row
                        nc.tensor.matmul(
                            vp_ps[:, 2 * jj, :],
                            lhsT=v_ld[0:C, j, :], rhs=wT["wv"][0:C, :],
                            start=True, stop=True)
                        for xh in range(2):  # hi: split (64,0)+M=128 HW bug
                            xr = slice(C * xh, C * (xh + 1))
                            nc.tensor.matmul(
                                vp_ps[xr, 2 * jj + 1, :],
                                lhsT=v_ld[C:128, j, xr],
                                rhs=wT["wv"][C:128, :],
                                start=True, stop=True)

                    # --- PSUM -> SBUF with fused biases ---
                    qkp = mid.tile([128, 4 * W], FP16, tag="qkp")
                    nc.scalar.activation(qkp[:, 0:2 * W], proj[:, 0:2 * W],
                                         AF.Identity, bias=bias2["bq"][:])
                    nc.scalar.activation(qkp[:, 2 * W:4 * W], proj[:, 2 * W:4 * W],
                                         AF.Identity, bias=bias2["bk"][:])
                    vp_s = mid.tile([128, 4, C], FP16, tag="vp_s")
                    nc.vector.tensor_add(
                        vp_s[:], vp_ps[:],
                        bv_bc[:].unsqueeze(1).broadcast_to((128, 4, C)))

                    # --- attention scores: attn_T[x, w] per row ---
                    attn_ps = psum.tile([128, 4, W], FP32, tag="ATTN")
                    for jj in range(2):
                        nc.tensor.matmul(
                            attn_ps[:, 2 * jj, :],
                            lhsT=qkp[0:C, (2 + jj) * W:(3 + jj) * W],
                            rhs=qkp[0:C, jj * W:(jj + 1) * W],
                            start=True, stop=True)
                        for xh in range(2):  # hi: split M=128 -> 2x M=64
                            nc.tensor.matmul(
                                attn_ps[C * xh:C * (xh + 1), 2 * jj + 1, :],
                                lhsT=qkp[C:128, (2 + jj) * W + C * xh:
                                         (2 + jj) * W + C * (xh + 1)],
                                rhs=qkp[C:128, jj * W:(jj + 1) * W],
                                start=True, stop=True)
                    attn_s = mid.tile([128, 4, W], FP16, tag="attn_s")
                    nc.scalar.activation(attn_s[:, 0:2, :], attn_ps[:, 0:2, :],
                                         AF.Copy)
                    nc.vector.tensor_copy(attn_s[:, 2:4, :], attn_ps[:, 2:4, :])

                    # --- output: res[c, w] per row, lo->top/hi->bottom ---
                    res_ps = psum.tile([128, 2, W], FP32, tag="RES")
                    for jj in range(2):
                        for lo in range(2):
                            pr = slice(C * lo, C * (lo + 1))
                            nc.tensor.matmul(
                                res_ps[pr, jj, :],
                                lhsT=vp_s[:, 2 * jj + lo, :],
                                rhs=attn_s[:, 2 * jj + lo, :],
                                start=True, stop=True)
                    # residual add (q was cast to fp16; fine vs |res|~500)
                    nc.vector.tensor_add(
                        out_ld[:, j0:j0 + 2, :], res_ps[:],
                        q_ld[:, j0:j0 + 2, :])

                nc.sync.dma_start(out=stacked(out_d, b, h0), in_=out_ld[:])

    nc.compile()  # bacc: legalize waits (<=1 per inst), reg alloc, DCE
    return nc


_NC_CACHE: dict = {}


def _get_nc() -> bass.Bass:
    if "nc" not in _NC_CACHE:
        _NC_CACHE["nc"] = build_nc()
    return _NC_CACHE["nc"]


def _make_in_maps(inputs: dict) -> list:
    f32 = lambda a: np.ascontiguousarray(np.asarray(a, dtype=np.float32))
    weights = {n: f32(inputs[n]) for n in ("wq", "bq", "wk", "bk", "wv", "bv")}
    in_maps = []
    for i in range(N_CORES):
        s = slice(i * BP, (i + 1) * BP)
        in_maps.append({
            "q": f32(inputs["q"][s]),
            "k": f32(inputs["k"][s]),
            "v": f32(inputs["v"][s]),
            **weights,
        })
    return in_maps


def kernel(**inputs) -> np.ndarray:
    nc = _get_nc()
    res = run_bass_kernel_spmd(nc, _make_in_maps(inputs), list(range(N_CORES)))
    return np.concatenate([r["out"] for r in res.results], axis=0)


# revision 11
# speedup vs baseline: 1.2504x; 1.2504x over previous
"""Trainium2 Bass kernel for unnormalized multi-head attention over W axis.

Reference computation (per batch b, row h):
    Qp[w,o] = sum_c wq[o,c] q[b,c,h,w] + bq[o]      (1x1 conv projections)
    Kp, Vp analogous
    attn[w,x] = sum_o Qp[w,o] Kp[x,o]
    out[b,o,h,w] = sum_x attn[w,x] Vp[x,o] + q[b,o,h,w]

Sharding: data-parallel over batch B=32 across 8 cores (4 batches/core),
weights replicated.

Layout: tiles of 16 attention rows are loaded "stacked": SBUF partitions
0-63 hold channels of rows h0..h0+7 ("lo"), partitions 64-127 hold rows
h0+8..h0+15 ("hi").  Every matmul runs as a lo/hi pair packed into
disjoint halves of the 128x128 PE array, doubling TensorE throughput and
keeping every PSUM->SBUF copy a full-width 128-partition op.

Matmul operands are fp16 (PSUM accumulation stays fp32): 16-bit matmuls
get a separate LDWEIGHTS instruction that pipelines with MATMULs through
the PE reorder window, roughly halving TensorE time vs fp32 self-loading
matmuls; inputs are cast fp32->fp16 by the (gpsimd) DMA itself.

HW bug workaround: a stationary loaded to PE rows 64-127 with M=128
(tile_position (64,0), all PE columns) crashes the exec unit, for any
dtype.  All "hi" matmuls with M=128 are split into two M=64 matmuls.
"""

from contextlib import ExitStack

import numpy as np

import concourse.bass as bass
import concourse.tile as tile
from concourse import bacc, mybir
from concourse.bass_utils import run_bass_kernel_spmd
from concourse.masks import make_identity

FP32 = mybir.dt.float32
FP16 = mybir.dt.float16
AF = mybir.ActivationFunctionType

B, C, H, W = 32, 64, 128, 128
N_CORES = 8
BP = B // N_CORES      # batches per core
GL = 32                # attention rows per load tile
NJ = GL // 2           # rows per half within a load tile
G = 4                  # rows per compute sub-group (2 pairs)


def build_nc(bp: int = BP, h: int = H, reps: int = 1) -> bass.Bass:
    """reps>1 repeats the whole (idempotent) computation for timing:
    per-iteration time = (wall[reps=R] - wall[reps=1]) / (R - 1)."""
    nc = bacc.Bacc(None, name="mha_w")

    q_d = nc.dram_tensor("q", [bp, C, h, W], FP32, kind="ExternalInput")
    k_d = nc.dram_tensor("k", [bp, C, h, W], FP32, kind="ExternalInput")
    v_d = nc.dram_tensor("v", [bp, C, h, W], FP32, kind="ExternalInput")
    wq_d = nc.dram_tensor("wq", [C, C], FP32, kind="ExternalInput")
    bq_d = nc.dram_tensor("bq", [C], FP32, kind="ExternalInput")
    wk_d = nc.dram_tensor("wk", [C, C], FP32, kind="ExternalInput")
    bk_d = nc.dram_tensor("bk", [C], FP32, kind="ExternalInput")
    wv_d = nc.dram_tensor("wv", [C, C], FP32, kind="ExternalInput")
    bv_d = nc.dram_tensor("bv", [C], FP32, kind="ExternalInput")
    out_d = nc.dram_tensor("out", [bp, C, h, W], FP32, kind="ExternalOutput")

    n_gl = h // GL
    hj = NJ

    with tile.TileContext(nc) as tc, ExitStack() as ctx:
        consts = ctx.enter_context(tc.tile_pool(name="consts", bufs=1))
        xin = ctx.enter_context(tc.tile_pool(name="xin", bufs=4))
        mid = ctx.enter_context(tc.tile_pool(name="mid", bufs=3))
        outs = ctx.enter_context(tc.tile_pool(name="outs", bufs=3))
        psum = ctx.enter_context(tc.tile_pool(name="psum", bufs=2, space="PSUM"))

        # ---- constants -------------------------------------------------
        i64 = consts.tile([C, C], FP32, tag="i64")
        make_identity(nc, i64)

        # transposed fp16 conv weights, duplicated on both partition halves
        wT = {}
        for name, wd in (("wq", wq_d), ("wk", wk_d), ("wv", wv_d)):
            w_nat = consts.tile([C, C], FP32, tag=f"{name}_nat")
            nc.sync.dma_start(out=w_nat[:], in_=wd[:, :])
            w_ps = psum.tile([128, C], FP32, tag="PROJ")
            nc.tensor.matmul(w_ps[0:C, :], lhsT=w_nat[:], rhs=i64[:],
                             start=True, stop=True)
            nc.tensor.matmul(w_ps[C:128, :], lhsT=w_nat[:], rhs=i64[:],
                             start=True, stop=True)
            wt = consts.tile([128, C], FP16, tag=f"{name}T")
            nc.vector.tensor_copy(wt[:], w_ps[:])
            wT[name] = wt

        # per-partition biases for q/k projections, on both halves (fp32)
        bias2 = {}
        for name, bd in (("bq", bq_d), ("bk", bk_d)):
            bt = consts.tile([128, 1], FP32, tag=f"{name}2")
            nc.sync.dma_start(out=bt[0:C, :], in_=bd[:].unsqueeze(1))
            nc.sync.dma_start(out=bt[C:128, :], in_=bd[:].unsqueeze(1))
            bias2[name] = bt

        # bv broadcast over all 128 partitions: [128, C] fp32
        bv_bc = consts.tile([128, C], FP32, tag="bv_bc")
        bv_bcast_ap = bass.AP(tensor=bv_d[:].tensor, offset=0,
                              ap=[[0, 128], [1, C]])
        nc.sync.dma_start(out=bv_bc[:], in_=bv_bcast_ap)

        # ---- main loop -------------------------------------------------
        def stacked(dram, b, h0):
            # [C, GL, W] -> (half, c, j, w), contiguous (j w) runs of 4KB
            a = dram[b, :, h0:h0 + GL, :]
            return a.rearrange("c (p j) w -> c p j w", p=2).transpose([1, 0, 2, 3])

        for rep in range(reps):
          for b in range(bp):
            for gl in range(n_gl):
                h0 = gl * GL
                q_ld = xin.tile([128, hj, W], FP16, tag="q_ld")
                k_ld = xin.tile([128, hj, W], FP16, tag="k_ld")
                v_ld = xin.tile([128, hj, W], FP16, tag="v_ld")
                # gpsimd DMA casts fp32 (DRAM) -> fp16 (SBUF)
                nc.gpsimd.dma_start(out=q_ld[:], in_=stacked(q_d, b, h0))
                nc.gpsimd.dma_start(out=k_ld[:], in_=stacked(k_d, b, h0))
                nc.gpsimd.dma_start(out=v_ld[:], in_=stacked(v_d, b, h0))

                out_ld = outs.tile([128, hj, W], FP32, tag="out_ld")

                for gc in range(hj // 2):  # sub-groups of 2 pairs = 4 rows
                    j0 = 2 * gc
                    # --- projections -> PSUM ---
                    proj = psum.tile([128, 4 * W], FP32, tag="PROJ")
                    vp_ps = psum.tile([128, 4, C], FP32, tag="VP")
                    for lo in range(2):
                        pr = slice(C * lo, C * (lo + 1))
                        # q/k: both row-pairs in one N=256 matmul
                        nc.tensor.matmul(
                            proj[pr, 0:2 * W],
                            lhsT=wT["wq"][pr, :], rhs=q_ld[pr, j0:j0 + 2, :],
                            start=True, stop=True)
                        nc.tensor.matmul(
                            proj[pr, 2 * W:4 * W],
                            lhsT=wT["wk"][pr, :], rhs=k_ld[pr, j0:j0 + 2, :],
                            start=True, stop=True)
                    for jj in range(2):
                        j = j0 + jj
                        # v projection, transposed: [x, c] per row
                        nc.tensor.matmul(
                            vp_ps[:, 2 * jj, :],
                            lhsT=v_ld[0:C, j, :], rhs=wT["wv"][0:C, :],
                            start=True, stop=True)
                        for xh in range(2):  # hi: split (64,0)+M=128 HW bug
                            xr = slice(C * xh, C * (xh + 1))
                            nc.tensor.matmul(
                                vp_ps[xr, 2 * jj + 1, :],
                                lhsT=v_ld[C:128, j, xr],
                                rhs=wT["wv"][C:128, :],
                                start=True, stop=True)

                    # --- PSUM -> SBUF with fused biases ---
                    qkp = mid.tile([128, 4 * W], FP16, tag="qkp")
                    nc.scalar.activation(qkp[:, 0:2 * W], proj[:, 0:2 * W],
                                         AF.Identity, bias=bias2["bq"][:])
                    nc.scalar.activation(qkp[:, 2 * W:4 * W], proj[:, 2 * W:4 * W],
                                         AF.Identity, bias=bias2["bk"][:])
                    vp_s = mid.tile([128, 4, C], FP16, tag="vp_s")
                    nc.vector.tensor_add(
                        vp_s[:], vp_ps[:],
                        bv_bc[:].unsqueeze(1).broadcast_to((128, 4, C)))

                    # --- attention scores: attn_T[x, w] per row ---
                    attn_ps = psum.tile([128, 4, W], FP32, tag="ATTN")
                    for jj in range(2):
                        nc.tensor.matmul(
                            attn_ps[:, 2 * jj, :],
                            lhsT=qkp[0:C, (2 + jj) * W:(3 + jj) * W],
                            rhs=qkp[0:C, jj * W:(jj + 1) * W],
                            start=True, stop=True)
                        for xh in range(2):  # hi: split M=128 -> 2x M=64
                            nc.tensor.matmul(
                                attn_ps[C * xh:C * (xh + 1), 2 * jj + 1, :],
                                lhsT=qkp[C:128, (2 + jj) * W + C * xh:
                                         (2 + jj) * W + C * (xh + 1)],
                                rhs=qkp[C:128, jj * W:(jj + 1) * W],
                                start=True, stop=True)
                    attn_s = mid.tile([128, 4, W], FP16, tag="attn_s")
                    nc.scalar.activation(attn_s[:, 0:2, :], attn_ps[:, 0:2, :],
                                         AF.Copy)
                    nc.vector.tensor_copy(attn_s[:, 2:4, :], attn_ps[:, 2:4, :])

                    # --- output: res[c, w] per row, lo->top/hi->bottom ---
                    res_ps = psum.tile([128, 2, W], FP32, tag="RES")
                    for jj in range(2):
                        for lo in range(2):
                            pr = slice(C * lo, C * (lo + 1))
                            nc.tensor.matmul(
                                res_ps[pr, jj, :],
                                lhsT=vp_s[:, 2 * jj + lo, :],
                                rhs=attn_s[:, 2 * jj + lo, :],
                                start=True, stop=True)
                    # residual add (q was cast to fp16; fine vs |res|~500)
                    nc.vector.tensor_add(
                        out_ld[:, j0:j0 + 2, :], res_ps[:],
                        q_ld[:, j0:j0 + 2, :])

                nc.sync.dma_start(out=stacked(out_d, b, h0), in_=out_ld[:])

    nc.compile()  # bacc: legalize waits (<=1 per inst), reg alloc, DCE
    return nc


_NC_CACHE: dict = {}


def _get_nc() -> bass.Bass:
    if "nc" not in _NC_CACHE:
        _NC_CACHE["nc"] = build_nc()
    return _NC_CACHE["nc"]


def _make_in_maps(inputs: dict) -> list:
    f32 = lambda a: np.ascontiguousarray(np.asarray(a, dtype=np.float32))
    weights = {n: f32(inputs[n]) for n in ("wq", "bq", "wk", "bk", "wv", "bv")}
    in_maps = []
    for i in range(N_CORES):
        s = slice(i * BP, (i + 1) * BP)
        in_maps.append({
            "q": f32(inputs["q"][s]),
            "k": f32(inputs["k"][s]),
            "v": f32(inputs["v"][s]),
            **weights,
        })
    return in_maps


def kernel(**inputs) -> np.ndarray:
    nc = _get_nc()
    res = run_bass_kernel_spmd(nc, _make_in_maps(inputs), list(range(N_CORES)))
    return np.concatenate([r["out"] for r in res.results], axis=0)
